# revision 18
# baseline (speedup 1.0000x reference)
"""Trainium2 Bass kernel for multi-head attention with RoPE.

Problem: b=8, n=1024, d_model=768, heads=12, dim_head=64.
Strategy: data parallel over batch — each of the 8 NeuronCores handles one
batch element end-to-end (QKV proj + RoPE + attention + out proj). No
collectives needed.

Per-core math (all in transposed [feature, token] layout so every matmul
contraction sits on the partition axis; all matmul operands padded to the
full 128 partitions for full SBUF-stream bandwidth):
  xT   [768,1024]  = x^T             (bf16, via DMA transpose)
  qT   [768,1024]  = Wq^T x^T        then RoPE in bf16 on DVE
  kz   2x[128,1024] per head pair: rotated k rows zero-padded to K=128
  V    [1024,12*128] = x Wv, 128 cols/head: 64 v | ones col | zeros
  per head h (single-head steps, software-pipelined one step ahead):
    sT[j,i] = sum_d kz[d,j] qT[d,i]  (K=128 contraction, zeros inert)
    pT  = exp(sT / 8)                (no max-subtraction; |S/8| <~ 6)
    oT[128,1024] += V-augmented accum over j tiles; row 64 = softmax
                    denominators (ones column), rows 65+ zeros
    aT-half = oT[0:64] * bcast(1/oT[64])  (recip_approx + DRAM-broadcast)
  out [1024,768] = aT^T Wout + b

v5 over v4 (same math; trace-driven restructure — the attention phase is
hard-floored by the ACT exp stream at ~1.0-1.2us per [128,1024] tile, so
everything else is scheduled to never stall that stream):
  - startup DMAs split into half-tiles so the first V matmul starts ~2us
    in (was ~11us): x row-tile halves interleave with the wv chunk each
    k-accumulation consumes; weights for later phases ride a second ring;
  - ALL 12 q/k projection tiles are emitted before attention in pair-need
    order (the v4 "filler" deferral of pair 5 into the attention phase
    stalled the exp stream ~7us for psum-slot reasons);
  - attention runs per-head (not per-pair): S double-buffer (4 psum banks)
    + one PV accumulator per head x2 in flight (4 banks) = all 8 banks;
    at head boundaries the next head's first S is emitted BEFORE the
    current head's two trailing PVs, so the exp stream never waits on a
    matmul that is queued behind PV work (~0.8us/boundary in v4);
  - per-head normalize: denominator row extract runs on the otherwise-idle
    Pool engine, the attn-rows copy on DVE, and the reciprocal broadcast
    round-trips through DRAM on the Pool DGE ring — nothing touches the
    ACT engine, which does exps back-to-back for the whole phase;
  - the exp activation table is pre-loaded by a tiny warmup exp during the
    V phase so exp #0 doesn't eat the ~1.5us table load;
  - the out-projection tail drops v4's fp32 ones-matmul broadcast (8 slow
    fp32 matmuls on the critical PE queue) for the same DRAM broadcast as
    every other head; e<5 batches interleave with the last head's
    normalize chain exactly as in v4.
"""

import os
import numpy as np
import ml_dtypes

N = 1024
D = 768
H = 12
DH = 64
E3 = 2304
KT = 6          # number of 128-row tiles of the model dim (768/128)
NT = 8          # number of 128-token tiles (1024/128)
P = 128
N_CORES = 8

_CACHE = {}


def _build():
    import concourse.bass as bass
    import concourse.mybir as mybir
    import concourse.tile as tile
    from concourse import bacc

    F32 = mybir.dt.float32
    BF16 = mybir.dt.bfloat16
    Exp = mybir.ActivationFunctionType.Exp

    nc = bacc.Bacc("TRN2", target_bir_lowering=False, debug=False,
                   num_devices=N_CORES)

    x = nc.dram_tensor("x", [D, N], BF16, kind="ExternalInput")
    wqkv = nc.dram_tensor("wqkv", [P, KT * 1536], BF16, kind="ExternalInput")
    wv_d = nc.dram_tensor("wv_d", [P, KT * D], BF16, kind="ExternalInput")
    wout = nc.dram_tensor("wout", [P, KT * D], BF16, kind="ExternalInput")
    cos2 = nc.dram_tensor("cos2", [P, N], BF16, kind="ExternalInput")
    sins2 = nc.dram_tensor("sins2", [P, N], BF16, kind="ExternalInput")
    # bias as [p, t]: per-partition column per output dcol-tile (the out
    # projection runs transposed, so bias is a per-partition ACT add)
    biasb = nc.dram_tensor("biasb", [P, KT], F32, kind="ExternalInput")
    # transposed output [d_model, tokens]; host transposes back
    out = nc.dram_tensor("out", [D, N], BF16, kind="ExternalOutput")

    with tile.TileContext(nc, pool_alloc_mode="queue") as tc:
        import contextlib
        with contextlib.ExitStack() as ctx:
            persist = ctx.enter_context(tc.tile_pool(name="persist", bufs=1))
            scr = ctx.enter_context(tc.tile_pool(name="scr", bufs=3))
            ptp = ctx.enter_context(tc.tile_pool(name="ptp", bufs=4))
            otp = ctx.enter_context(tc.tile_pool(name="otp", bufs=2))
            dnp = ctx.enter_context(tc.tile_pool(name="dnp", bufs=2))
            rcp = ctx.enter_context(tc.tile_pool(name="rcp", bufs=2))
            rbp = ctx.enter_context(tc.tile_pool(name="rbp", bufs=2))
            outp = ctx.enter_context(tc.tile_pool(name="outp", bufs=3))
            dramp = ctx.enter_context(
                tc.tile_pool(name="dram", bufs=2, space="DRAM"))

            # ---- startup loads. Ring A (sync DGE): what the V projection
            # consumes, in consumption order — for each k, the front half
            # of the x row-tile plus the wv chunk, so the ni<4 matmuls
            # chase arrivals; back halves next; then q/k weights + rope
            # tables (needed ~15us in). Ring B (Pool DGE): tail-only
            # weights (w_out, bias).
            xT = [persist.tile([P, N], BF16, tag=f"xT{t_i}",
                               name=f"xT_sb{t_i}") for t_i in range(KT)]
            wv_sb = persist.tile([P, KT * D], BF16, tag="wv", name="wv_sb")
            HN = N // 2
            for t_i in range(KT):
                nc.sync.dma_start(xT[t_i][:, 0:HN], x[t_i * P:(t_i + 1) * P, 0:HN])
                nc.sync.dma_start(wv_sb[:, t_i * D:(t_i + 1) * D],
                                  wv_d[:, t_i * D:(t_i + 1) * D])
            # w_qkv is host-relaid m-contiguous ([128, (m k 128)]) so each
            # projection tile's weights are one 192KB chunk; the first qk
            # tiles' chunks land before the x back-halves so the qk phase
            # is never DMA-gated.
            wqk_sb = persist.tile([P, KT * 1536], BF16, tag="wqk",
                                  name="wqk_sb")
            for m in (0, 6, 1, 7):
                nc.sync.dma_start(wqk_sb[:, m * D:(m + 1) * D],
                                  wqkv[:, m * D:(m + 1) * D])
            for t_i in range(KT):
                nc.sync.dma_start(xT[t_i][:, HN:N], x[t_i * P:(t_i + 1) * P, HN:N])
            for m in (2, 8, 3, 9, 4, 10, 5, 11):
                nc.sync.dma_start(wqk_sb[:, m * D:(m + 1) * D],
                                  wqkv[:, m * D:(m + 1) * D])
            cos_sb = persist.tile([P, N], BF16, tag="cos", name="cos_sb")
            nc.sync.dma_start(cos_sb[:], cos2[:, :])
            sin_sb = persist.tile([P, N], BF16, tag="sin", name="sin_sb")
            nc.sync.dma_start(sin_sb[:], sins2[:, :])
            wo_sb = persist.tile([P, KT * D], BF16, tag="wo", name="wo_sb")
            nc.sync.dma_start(wo_sb[:], wout[:, :])
            bias_sb = persist.tile([P, KT], F32, tag="bias", name="bias_sb")
            nc.sync.dma_start(bias_sb[:], biasb[:, :])

            qkT = [persist.tile([P, N], BF16, tag=f"qkT{m}", name=f"qkT_sb{m}")
                   for m in range(6)]
            # zero-padded K tiles: kz[hp][u] holds head 2hp+u's rotated k
            # rows in their natural 64-row half, zeros in the other half,
            # so S^T matmuls contract a full K=128 (full SBUF stream BW).
            kz = [[persist.tile([P, N], BF16, tag=f"kz{hp}_{u}",
                                name=f"kz_sb{hp}_{u}") for u in range(2)]
                  for hp in range(6)]
            # V tiles padded to 128 cols/head: 64 v-dims | ones | zeros,
            # so PV matmuls write a full M=128 (ones col -> sums row 64).
            vt = [persist.tile([P, H * P], BF16, tag=f"vt{n}", name=f"vt_sb{n}")
                  for n in range(NT)]
            aT = [persist.tile([P, N], BF16, tag=f"aT{e}", name=f"aT_sb{e}")
                  for e in range(KT)]
            for hp in range(6):
                nc.gpsimd.memset(kz[hp][0][DH:P, :], 0.0)
                nc.gpsimd.memset(kz[hp][1][0:DH, :], 0.0)

            # warm the ACT Exp table while the engine is idle so the first
            # real exp of the attention phase doesn't pay the table load
            warm_in = persist.tile([1, 8], F32, tag="wrmi", name="warm_in")
            warm_out = persist.tile([1, 8], BF16, tag="wrmo", name="warm_out")
            nc.gpsimd.memset(warm_in[:], 0.0)
            nc.scalar.activation(warm_out[:], warm_in[:], Exp, scale=0.125)

            # Two dedicated PSUM pools (2 slots x 2 banks each = all 8
            # banks): psS cycles projection / S^T / final tiles, psO holds
            # the in-flight PV accumulators (one per head, two heads deep).
            with (tc.tile_pool(name="psS", bufs=2, space="PSUM") as psum,
                  tc.tile_pool(name="psO", bufs=2, space="PSUM") as psumO):
                # ---- V projection into per-head 65-wide layout ----
                for ni in range(NT):
                    vpool, vtag = ((psum, "ps") if ni % 2 == 0
                                   else (psumO, "ops"))
                    ps = vpool.tile([P, N], F32, tag=vtag, name="ps_v")
                    # k-major: both column chunks share one stationary
                    # operand per k (weight load hides fully)
                    for k in range(KT):
                        for (c0, cw) in ((0, 512), (512, 256)):
                            nc.tensor.matmul(
                                ps[:, c0:c0 + cw],
                                lhsT=xT[k][:, ni * P:(ni + 1) * P],
                                rhs=wv_sb[:, k * D + c0:k * D + c0 + cw],
                                start=(k == 0), stop=(k == KT - 1))
                    # scatter copy into head-strided slots; split across
                    # ACT (8-head chunk) and DVE (4-head chunk) so neither
                    # engine gates the psum slot rotation
                    dst8 = vt[ni][:, 0:8 * P].rearrange(
                        "p (h j) -> p h j", j=P)[:, :, 0:DH]
                    src8 = ps[:, 0:512].rearrange("p (h j) -> p h j", j=DH)
                    nc.scalar.copy(dst8, src8)
                    dst4 = vt[ni][:, 8 * P:12 * P].rearrange(
                        "p (h j) -> p h j", j=P)[:, :, 0:DH]
                    src4 = ps[:, 512:768].rearrange("p (h j) -> p h j", j=DH)
                    if ni >= NT - 2:
                        # last two tiles: all-ACT, so no DVE queue lag
                        # holds the psum slot the first qk tile needs
                        nc.scalar.copy(dst4, src4)
                    else:
                        nc.vector.tensor_copy(dst4, src4)
                    vre = vt[ni].rearrange("p (h j) -> p h j", j=P)
                    nc.gpsimd.memset(vre[:, :, DH:DH + 1], 1.0)
                    nc.gpsimd.memset(vre[:, :, DH + 1:P], 0.0)

                # ---- q/k projection + RoPE, ALL 12 tiles before the
                # attention phase, in pair-need order (head-pair hp needs
                # tiles hp and 6+hp). The last tiles belong to pair 5,
                # whose attention steps run ~80us later — plenty of rope
                # latency slack.
                def emit_qk(m, qpool, qtag):
                    ps = qpool.tile([P, N], F32, tag=qtag, name="ps_qk")
                    # k-major: both ih matmuls share one stationary
                    # operand, so the weight load hides fully
                    for k in range(KT):
                        for ih in range(2):
                            nc.tensor.matmul(
                                ps[:, ih * 512:(ih + 1) * 512],
                                lhsT=wqk_sb[:, m * D + k * P:
                                            m * D + (k + 1) * P],
                                rhs=xT[k][:, ih * 512:(ih + 1) * 512],
                                start=(k == 0), stop=(k == KT - 1))
                    # RoPE in bf16. rotate-half via a 4-block row-swap
                    # DMA (sync DGE ring, triggered right after the qf
                    # drain) + ONE full-width DVE multiply — quarter
                    # width DVE ops cost a full column pass each.
                    qf = scr.tile([P, N], BF16, tag="qf", name="qf_t")
                    nc.scalar.copy(qf[:], ps[:])
                    qs = scr.tile([P, N], BF16, tag="qs", name="qs_t")
                    for blk in range(4):
                        ob = blk * 32
                        ib = (blk ^ 1) * 32  # 0<->32, 64<->96
                        nc.sync.dma_start(qs[ob:ob + 32, :],
                                          qf[ib:ib + 32, :])
                    qa = scr.tile([P, N], BF16, tag="qa", name="qa_t")
                    nc.vector.tensor_mul(qa[:], qf[:], cos_sb[:])
                    nc.vector.tensor_mul(qs[:], qs[:], sin_sb[:])
                    if m < 6:
                        nc.vector.tensor_add(qkT[m][:], qa[:], qs[:])
                    else:
                        hp = m - 6
                        nc.vector.tensor_add(
                            kz[hp][0][0:DH, :], qa[0:DH, :], qs[0:DH, :])
                        nc.vector.tensor_add(
                            kz[hp][1][DH:P, :], qa[DH:P, :], qs[DH:P, :])

                for mi, m in enumerate(
                        [t for hp in range(6) for t in (hp, 6 + hp)]):
                    if mi % 2 == 0:
                        emit_qk(m, psum, "ps")
                    else:
                        emit_qk(m, psumO, "ops")

                # ---- attention, one head at a time, software-pipelined
                # one step: S/exp of step g+1 is emitted before PV of step
                # g. At head boundaries the next head's first S moves
                # ahead of BOTH trailing PVs so the exp stream (the phase
                # bottleneck) never waits on queued PV matmuls.
                o_ps_box = [None] * H

                def emit_s_exp(h, j):
                    hp, u = h // 2, h % 2
                    s_ps = psum.tile([P, N], F32, tag="ps", name="s_ps")
                    for ih in range(2):
                        nc.tensor.matmul(
                            s_ps[:, ih * 512:(ih + 1) * 512],
                            lhsT=kz[hp][u][:, j * P:(j + 1) * P],
                            rhs=qkT[hp][:, ih * 512:(ih + 1) * 512],
                            start=True, stop=True)
                    pT = ptp.tile([P, N], BF16, tag="pT", name="pT_t")
                    nc.scalar.activation(pT[:], s_ps[:], Exp, scale=0.125)
                    return pT

                def emit_norm(h, o_ps):
                    # drain the accumulator (frees the psum slot):
                    # denominator row + attn rows copied out, then
                    # reciprocal + DRAM broadcast + scale. Copies ride
                    # DVE mid-spine (ACT must keep streaming exps); the
                    # LAST head's copies ride the now-idle ACT since that
                    # chain is the tail's critical path. The reciprocal
                    # stays partition-0-aligned on both sides (custom-DVE
                    # ops don't shift across partitions).
                    hp, u = h // 2, h % 2
                    den = dnp.tile([1, N], F32, tag="dn", name="den_t")
                    oTc = otp.tile([DH, N], BF16, tag="oT", name="oT_t")
                    if h == H - 1:
                        nc.scalar.copy(den[:], o_ps[DH:DH + 1, :])
                        nc.scalar.copy(oTc[:], o_ps[0:DH, :])
                    else:
                        nc.vector.tensor_copy(den[:], o_ps[DH:DH + 1, :])
                        nc.vector.tensor_copy(oTc[:], o_ps[0:DH, :])
                    r_sb = rcp.tile([1, N], F32, tag="r", name="r_t")
                    nc.vector.reciprocal_approx_fast(r_sb[:], den[:])
                    r_dr = dramp.tile([1, N], F32, tag="rdr", name="rdr_t")
                    nc.sync.dma_start(r_dr[:], r_sb[:])
                    rb_sb = rbp.tile([DH, N], F32, tag="rb", name="rb_t")
                    nc.sync.dma_start(rb_sb[:],
                                      r_dr[0:1, :].broadcast_to([DH, N]))
                    nc.vector.tensor_mul(aT[hp][u * DH:(u + 1) * DH, :],
                                         oTc[:], rb_sb[:])

                def emit_pv(h, j, pT):
                    if j == 0:
                        o_ps_box[h] = psumO.tile([P, N], F32, tag="ops",
                                                 name="o_ps")
                    o_ps = o_ps_box[h]
                    for ih in range(2):
                        nc.tensor.matmul(
                            o_ps[:, ih * 512:(ih + 1) * 512],
                            lhsT=vt[j][:, h * P:(h + 1) * P],
                            rhs=pT[:, ih * 512:(ih + 1) * 512],
                            start=(j == 0), stop=(j == NT - 1))
                    if j == NT - 1:
                        emit_norm(h, o_ps)

                # out-projection helpers (some batches weave into the
                # spine's tail slack, so defined before the loop)
                f_tiles = [None] * KT

                def emit_e04(t):
                    f_pool = psum if t % 2 == 1 else psumO
                    f_ps = f_pool.tile([P, N], F32,
                                       tag="ps" if t % 2 == 1 else "ops",
                                       name="f_ps")
                    f_tiles[t] = f_ps
                    for e in range(KT - 1):
                        for ih in range(2):
                            nc.tensor.matmul(
                                f_ps[:, ih * 512:(ih + 1) * 512],
                                lhsT=wo_sb[:, e * D + t * P:
                                           e * D + (t + 1) * P],
                                rhs=aT[e][:, ih * 512:(ih + 1) * 512],
                                start=(e == 0), stop=False)

                def emit_e5_store(t):
                    f_ps = f_tiles[t]
                    for ih in range(2):
                        nc.tensor.matmul(
                            f_ps[:, ih * 512:(ih + 1) * 512],
                            lhsT=wo_sb[:, 5 * D + t * P:5 * D + (t + 1) * P],
                            rhs=aT[5][:, ih * 512:(ih + 1) * 512],
                            start=False, stop=True)
                    o_sb = outp.tile([P, N], BF16, tag="osb", name="osb_t")
                    nc.scalar.add(o_sb[:], f_ps[:], bias_sb[:, t:t + 1])
                    nc.sync.dma_start(out[t * P:(t + 1) * P, :], o_sb[:])

                # 2-step S lead: emit S/exp(g+2) before PV(g), so the S
                # matmuls feeding exp(g+2) run right after exp(g) frees
                # its psum slot instead of queueing behind PV(g) (which
                # itself waits on exp(g)) — keeps the ACT exp stream
                # gapless. PSUM: exactly two S tiles live at any time
                # (the one being exp'd + the one just written). The first
                # e<5 out-proj batch is emitted just before the LAST PV:
                # its psS slot freed at exp(94), so its matmuls fill the
                # PE slack under the final exps instead of the tail.
                steps = [(h, j) for h in range(H) for j in range(NT)]
                pT_q = [emit_s_exp(*steps[0]), emit_s_exp(*steps[1])]
                for gi, st in enumerate(steps):
                    if gi + 2 < len(steps):
                        pT_q.append(emit_s_exp(*steps[gi + 2]))
                    if gi == len(steps) - 1:
                        emit_e04(1)
                    emit_pv(st[0], st[1], pT_q.pop(0))

                # ---- output projection tail, TRANSPOSED: outT[dcol,
                # tok] = sum_e wo(e, dcol-tile)^T @ aT[e] (+ bias via
                # ACT, a per-partition add). e<5 batches first (only e=5
                # needs aT[5], whose head-11 normalize chain is still in
                # flight) — PE is strict FIFO, so a blocked e=5 matmul
                # would stall everything. Batch t=1 was emitted inside
                # the spine already.
                emit_e04(3)
                emit_e04(0)
                emit_e04(2)
                emit_e5_store(1)
                emit_e04(5)
                emit_e5_store(3)
                emit_e5_store(0)
                emit_e04(4)
                emit_e5_store(2)
                emit_e5_store(5)
                emit_e5_store(4)

    nc.compile()
    return nc


def _host_tables():
    inv_freq = 1.0 / (10000.0 ** (np.arange(0, DH, 2, dtype=np.float32) / DH))
    t = np.arange(N, dtype=np.float32)
    freqs = np.einsum("i,j->ij", t, inv_freq)          # [N, 32]
    emb = np.concatenate([freqs, freqs], axis=-1)      # [N, 64]
    cosT = np.cos(emb).T.astype(np.float32)            # [64, N]
    sinT = np.sin(emb).T.astype(np.float32)            # [64, N]
    # b-term: out rows 0:32 use -sin (pair d+32), rows 32:64 use +sin.
    # No pre-swap: the device block-swaps qf itself (row-swap DMA), so
    # the sin table stays in output-row order.
    sins = np.concatenate([-sinT[0:32], sinT[32:64]], axis=0)  # [64, N]
    cos2 = np.concatenate([cosT, cosT], axis=0)        # [128, N]
    sins2 = np.concatenate([sins, sins], axis=0)       # [128, N]
    return np.ascontiguousarray(cos2), np.ascontiguousarray(sins2)


def kernel(x, w_qkv, w_out, b_out):
    from concourse.bass_utils import run_bass_kernel_spmd

    if "nc" not in _CACHE:
        _CACHE["nc"] = _build()
    nc = _CACHE["nc"]

    bf = ml_dtypes.bfloat16
    cos2, sins2 = _host_tables()
    cos2 = np.ascontiguousarray(cos2.astype(bf))
    sins2 = np.ascontiguousarray(sins2.astype(bf))
    # [p, t]: bias value for output row t*128+p (transposed out proj)
    biasb = np.ascontiguousarray(
        np.asarray(b_out, np.float32).reshape(KT, P).T)

    def _sbufize(w):   # [(k p), e] -> [p, (k e)] exact SBUF layout
        w = np.asarray(w, np.float32).astype(bf)
        k, e = w.shape[0] // P, w.shape[1]
        return np.ascontiguousarray(
            w.reshape(k, P, e).transpose(1, 0, 2).reshape(P, k * e))

    # q/k weights m-contiguous: [p, (m k 128)] where m = 128-col output
    # tile (12 of them), k = contraction tile — one DMA chunk per m-tile
    wqk_ke = _sbufize(np.asarray(w_qkv, np.float32)[:, 0:1536])  # [p,(k m 128)]
    wqkv_b = np.ascontiguousarray(
        wqk_ke.reshape(P, KT, 12, P).transpose(0, 2, 1, 3).reshape(P, KT * 1536))
    wv_b = _sbufize(np.asarray(w_qkv, np.float32)[:, 1536:E3])
    wout_b = _sbufize(w_out)

    in_maps = []
    for i in range(N_CORES):
        xi = np.ascontiguousarray(
            np.asarray(x[i], np.float32).astype(bf).T)
        in_maps.append({
            "x": xi, "wqkv": wqkv_b, "wv_d": wv_b, "wout": wout_b,
            "cos2": cos2, "sins2": sins2, "biasb": biasb,
        })

    res = run_bass_kernel_spmd(
        nc, in_maps, list(range(N_CORES)),
        trace=bool(int(os.environ.get("KERNEL_TRACE", "0"))))
    _CACHE["last_result"] = res
    return np.stack([np.asarray(res.results[i]["out"]).T
                     for i in range(N_CORES)], axis=0).astype(np.float32)


# revision 22
# speedup vs baseline: 1.0125x; 1.0125x over previous
"""Trainium2 Bass kernel for multi-head attention with RoPE.

Problem: b=8, n=1024, d_model=768, heads=12, dim_head=64.
Strategy: data parallel over batch — each of the 8 NeuronCores handles one
batch element end-to-end (QKV proj + RoPE + attention + out proj). No
collectives needed.

Per-core math (all in transposed [feature, token] layout so every matmul
contraction sits on the partition axis; all matmul operands padded to the
full 128 partitions for full SBUF-stream bandwidth):
  xT   [768,1024]  = x^T             (bf16, via DMA transpose)
  qT   [768,1024]  = Wq^T x^T        then RoPE in bf16 on DVE
  kz   2x[128,1024] per head pair: rotated k rows zero-padded to K=128
  V    [1024,12*128] = x Wv, 128 cols/head: 64 v | ones col | zeros
  per head h (single-head steps, software-pipelined one step ahead):
    sT[j,i] = sum_d kz[d,j] qT[d,i]  (K=128 contraction, zeros inert)
    pT  = exp(sT / 8)                (no max-subtraction; |S/8| <~ 6)
    oT[128,1024] += V-augmented accum over j tiles; row 64 = softmax
                    denominators (ones column), rows 65+ zeros
    aT-half = oT[0:64] * bcast(1/oT[64])  (recip_approx + DRAM-broadcast)
  out [1024,768] = aT^T Wout + b

v5 over v4 (same math; trace-driven restructure — the attention phase is
hard-floored by the ACT exp stream at ~1.0-1.2us per [128,1024] tile, so
everything else is scheduled to never stall that stream):
  - startup DMAs split into half-tiles so the first V matmul starts ~2us
    in (was ~11us): x row-tile halves interleave with the wv chunk each
    k-accumulation consumes; weights for later phases ride a second ring;
  - ALL 12 q/k projection tiles are emitted before attention in pair-need
    order (the v4 "filler" deferral of pair 5 into the attention phase
    stalled the exp stream ~7us for psum-slot reasons);
  - attention runs per-head (not per-pair): S double-buffer (4 psum banks)
    + one PV accumulator per head x2 in flight (4 banks) = all 8 banks;
    at head boundaries the next head's first S is emitted BEFORE the
    current head's two trailing PVs, so the exp stream never waits on a
    matmul that is queued behind PV work (~0.8us/boundary in v4);
  - per-head normalize: denominator row extract runs on the otherwise-idle
    Pool engine, the attn-rows copy on DVE, and the reciprocal broadcast
    round-trips through DRAM on the Pool DGE ring — nothing touches the
    ACT engine, which does exps back-to-back for the whole phase;
  - the exp activation table is pre-loaded by a tiny warmup exp during the
    V phase so exp #0 doesn't eat the ~1.5us table load;
  - the out-projection tail drops v4's fp32 ones-matmul broadcast (8 slow
    fp32 matmuls on the critical PE queue) for the same DRAM broadcast as
    every other head; e<5 batches interleave with the last head's
    normalize chain exactly as in v4.
"""

import os
import numpy as np
import ml_dtypes

N = 1024
D = 768
H = 12
DH = 64
E3 = 2304
KT = 6          # number of 128-row tiles of the model dim (768/128)
NT = 8          # number of 128-token tiles (1024/128)
P = 128
N_CORES = 8

_CACHE = {}


def _build():
    import concourse.bass as bass
    import concourse.mybir as mybir
    import concourse.tile as tile
    from concourse import bacc

    F32 = mybir.dt.float32
    BF16 = mybir.dt.bfloat16
    Exp = mybir.ActivationFunctionType.Exp

    nc = bacc.Bacc("TRN2", target_bir_lowering=False, debug=False,
                   num_devices=N_CORES)

    x = nc.dram_tensor("x", [D, N], BF16, kind="ExternalInput")
    wqkv = nc.dram_tensor("wqkv", [P, KT * 1536], BF16, kind="ExternalInput")
    wv_d = nc.dram_tensor("wv_d", [P, KT * D], BF16, kind="ExternalInput")
    wout = nc.dram_tensor("wout", [P, KT * D], BF16, kind="ExternalInput")
    cos2 = nc.dram_tensor("cos2", [P, N], BF16, kind="ExternalInput")
    sins2 = nc.dram_tensor("sins2", [P, N], BF16, kind="ExternalInput")
    # bias as [p, t]: per-partition column per output dcol-tile (the out
    # projection runs transposed, so bias is a per-partition ACT add)
    biasb = nc.dram_tensor("biasb", [P, KT], F32, kind="ExternalInput")
    # transposed output [d_model, tokens]; host transposes back
    out = nc.dram_tensor("out", [D, N], BF16, kind="ExternalOutput")

    with tile.TileContext(nc, pool_alloc_mode="queue") as tc:
        import contextlib
        with contextlib.ExitStack() as ctx:
            persist = ctx.enter_context(tc.tile_pool(name="persist", bufs=1))
            scr = ctx.enter_context(tc.tile_pool(name="scr", bufs=3))
            ptp = ctx.enter_context(tc.tile_pool(name="ptp", bufs=4))
            otp = ctx.enter_context(tc.tile_pool(name="otp", bufs=2))
            dnp = ctx.enter_context(tc.tile_pool(name="dnp", bufs=2))
            rcp = ctx.enter_context(tc.tile_pool(name="rcp", bufs=2))
            rbp = ctx.enter_context(tc.tile_pool(name="rbp", bufs=2))
            outp = ctx.enter_context(tc.tile_pool(name="outp", bufs=3))
            dramp = ctx.enter_context(
                tc.tile_pool(name="dram", bufs=2, space="DRAM"))

            # ---- startup loads. Ring A (sync DGE): what the V projection
            # consumes, in consumption order — for each k, the front half
            # of the x row-tile plus the wv chunk, so the ni<4 matmuls
            # chase arrivals; back halves next; then q/k weights + rope
            # tables (needed ~15us in). Ring B (Pool DGE): tail-only
            # weights (w_out, bias).
            xT = [persist.tile([P, N], BF16, tag=f"xT{t_i}",
                               name=f"xT_sb{t_i}") for t_i in range(KT)]
            wv_sb = persist.tile([P, KT * D], BF16, tag="wv", name="wv_sb")
            HN = N // 2
            for t_i in range(KT):
                nc.sync.dma_start(xT[t_i][:, 0:HN], x[t_i * P:(t_i + 1) * P, 0:HN])
                nc.sync.dma_start(wv_sb[:, t_i * D:(t_i + 1) * D],
                                  wv_d[:, t_i * D:(t_i + 1) * D])
            # w_qkv is host-relaid m-contiguous ([128, (m k 128)]) so each
            # projection tile's weights are one 192KB chunk; the first qk
            # tiles' chunks land before the x back-halves so the qk phase
            # is never DMA-gated.
            wqk_sb = persist.tile([P, KT * 1536], BF16, tag="wqk",
                                  name="wqk_sb")
            for m in (0, 6, 1, 7):
                nc.sync.dma_start(wqk_sb[:, m * D:(m + 1) * D],
                                  wqkv[:, m * D:(m + 1) * D])
            for t_i in range(KT):
                nc.sync.dma_start(xT[t_i][:, HN:N], x[t_i * P:(t_i + 1) * P, HN:N])
            for m in (2, 8, 3, 9, 4, 10, 5, 11):
                nc.sync.dma_start(wqk_sb[:, m * D:(m + 1) * D],
                                  wqkv[:, m * D:(m + 1) * D])
            cos_sb = persist.tile([P, N], BF16, tag="cos", name="cos_sb")
            nc.sync.dma_start(cos_sb[:], cos2[:, :])
            sin_sb = persist.tile([P, N], BF16, tag="sin", name="sin_sb")
            nc.sync.dma_start(sin_sb[:], sins2[:, :])
            wo_sb = persist.tile([P, KT * D], BF16, tag="wo", name="wo_sb")
            nc.sync.dma_start(wo_sb[:], wout[:, :])
            bias_sb = persist.tile([P, KT], F32, tag="bias", name="bias_sb")
            nc.sync.dma_start(bias_sb[:], biasb[:, :])

            qkT = [persist.tile([P, N], BF16, tag=f"qkT{m}", name=f"qkT_sb{m}")
                   for m in range(6)]
            # zero-padded K tiles: kz[hp][u] holds head 2hp+u's rotated k
            # rows in their natural 64-row half, zeros in the other half,
            # so S^T matmuls contract a full K=128 (full SBUF stream BW).
            kz = [[persist.tile([P, N], BF16, tag=f"kz{hp}_{u}",
                                name=f"kz_sb{hp}_{u}") for u in range(2)]
                  for hp in range(6)]
            # V tiles padded to 128 cols/head: 64 v-dims | ones | zeros,
            # so PV matmuls write a full M=128 (ones col -> sums row 64).
            vt = [persist.tile([P, H * P], BF16, tag=f"vt{n}", name=f"vt_sb{n}")
                  for n in range(NT)]
            aT = [persist.tile([P, N], BF16, tag=f"aT{e}", name=f"aT_sb{e}")
                  for e in range(KT)]
            for hp in range(6):
                nc.gpsimd.memset(kz[hp][0][DH:P, :], 0.0)
                nc.gpsimd.memset(kz[hp][1][0:DH, :], 0.0)
            ones_sb = persist.tile([1, DH], F32, tag="ones", name="ones_t")
            nc.gpsimd.memset(ones_sb[:], 1.0)

            # warm the ACT Exp table while the engine is idle so the first
            # real exp of the attention phase doesn't pay the table load
            warm_in = persist.tile([1, 8], F32, tag="wrmi", name="warm_in")
            warm_out = persist.tile([1, 8], BF16, tag="wrmo", name="warm_out")
            nc.gpsimd.memset(warm_in[:], 0.0)
            nc.scalar.activation(warm_out[:], warm_in[:], Exp, scale=0.125)

            # Two dedicated PSUM pools (2 slots x 2 banks each = all 8
            # banks): psS cycles projection / S^T / final tiles, psO holds
            # the in-flight PV accumulators (one per head, two heads deep).
            with (tc.tile_pool(name="psS", bufs=2, space="PSUM") as psum,
                  tc.tile_pool(name="psO", bufs=2, space="PSUM") as psumO):
                # ---- V projection into per-head 65-wide layout ----
                for ni in range(NT):
                    vpool, vtag = ((psum, "ps") if ni % 2 == 0
                                   else (psumO, "ops"))
                    ps = vpool.tile([P, N], F32, tag=vtag, name="ps_v")
                    # k-major: both column chunks share one stationary
                    # operand per k (weight load hides fully)
                    for k in range(KT):
                        for (c0, cw) in ((0, 512), (512, 256)):
                            nc.tensor.matmul(
                                ps[:, c0:c0 + cw],
                                lhsT=xT[k][:, ni * P:(ni + 1) * P],
                                rhs=wv_sb[:, k * D + c0:k * D + c0 + cw],
                                start=(k == 0), stop=(k == KT - 1))
                    # scatter copy into head-strided slots; split across
                    # ACT (8-head chunk) and DVE (4-head chunk) so neither
                    # engine gates the psum slot rotation
                    dst8 = vt[ni][:, 0:8 * P].rearrange(
                        "p (h j) -> p h j", j=P)[:, :, 0:DH]
                    src8 = ps[:, 0:512].rearrange("p (h j) -> p h j", j=DH)
                    nc.scalar.copy(dst8, src8)
                    dst4 = vt[ni][:, 8 * P:12 * P].rearrange(
                        "p (h j) -> p h j", j=P)[:, :, 0:DH]
                    src4 = ps[:, 512:768].rearrange("p (h j) -> p h j", j=DH)
                    if ni >= NT - 2:
                        # last two tiles: all-ACT, so no DVE queue lag
                        # holds the psum slot the first qk tile needs
                        nc.scalar.copy(dst4, src4)
                    else:
                        nc.vector.tensor_copy(dst4, src4)
                    vre = vt[ni].rearrange("p (h j) -> p h j", j=P)
                    nc.gpsimd.memset(vre[:, :, DH:DH + 1], 1.0)
                    nc.gpsimd.memset(vre[:, :, DH + 1:P], 0.0)

                # ---- q/k projection + RoPE, ALL 12 tiles before the
                # attention phase, in pair-need order (head-pair hp needs
                # tiles hp and 6+hp). The last tiles belong to pair 5,
                # whose attention steps run ~80us later — plenty of rope
                # latency slack.
                def emit_qk(m, qpool, qtag):
                    ps = qpool.tile([P, N], F32, tag=qtag, name="ps_qk")
                    # k-major: both ih matmuls share one stationary
                    # operand, so the weight load hides fully
                    for k in range(KT):
                        for ih in range(2):
                            nc.tensor.matmul(
                                ps[:, ih * 512:(ih + 1) * 512],
                                lhsT=wqk_sb[:, m * D + k * P:
                                            m * D + (k + 1) * P],
                                rhs=xT[k][:, ih * 512:(ih + 1) * 512],
                                start=(k == 0), stop=(k == KT - 1))
                    # RoPE in bf16. rotate-half via a 4-block row-swap
                    # DMA (sync DGE ring, triggered right after the qf
                    # drain) + ONE full-width DVE multiply — quarter
                    # width DVE ops cost a full column pass each.
                    qf = scr.tile([P, N], BF16, tag="qf", name="qf_t")
                    nc.scalar.copy(qf[:], ps[:])
                    qs = scr.tile([P, N], BF16, tag="qs", name="qs_t")
                    for blk in range(4):
                        ob = blk * 32
                        ib = (blk ^ 1) * 32  # 0<->32, 64<->96
                        nc.sync.dma_start(qs[ob:ob + 32, :],
                                          qf[ib:ib + 32, :])
                    qa = scr.tile([P, N], BF16, tag="qa", name="qa_t")
                    nc.vector.tensor_mul(qa[:], qf[:], cos_sb[:])
                    nc.vector.tensor_mul(qs[:], qs[:], sin_sb[:])
                    if m < 6:
                        nc.vector.tensor_add(qkT[m][:], qa[:], qs[:])
                    else:
                        hp = m - 6
                        nc.vector.tensor_add(
                            kz[hp][0][0:DH, :], qa[0:DH, :], qs[0:DH, :])
                        nc.vector.tensor_add(
                            kz[hp][1][DH:P, :], qa[DH:P, :], qs[DH:P, :])

                for mi, m in enumerate(
                        [t for hp in range(6) for t in (hp, 6 + hp)]):
                    if mi % 2 == 0:
                        emit_qk(m, psum, "ps")
                    else:
                        emit_qk(m, psumO, "ops")

                # ---- attention, one head at a time, software-pipelined
                # one step: S/exp of step g+1 is emitted before PV of step
                # g. At head boundaries the next head's first S moves
                # ahead of BOTH trailing PVs so the exp stream (the phase
                # bottleneck) never waits on queued PV matmuls.
                o_ps_box = [None] * H

                def emit_s_exp(h, j):
                    hp, u = h // 2, h % 2
                    s_ps = psum.tile([P, N], F32, tag="ps", name="s_ps")
                    for ih in range(2):
                        nc.tensor.matmul(
                            s_ps[:, ih * 512:(ih + 1) * 512],
                            lhsT=kz[hp][u][:, j * P:(j + 1) * P],
                            rhs=qkT[hp][:, ih * 512:(ih + 1) * 512],
                            start=True, stop=True)
                    pT = ptp.tile([P, N], BF16, tag="pT", name="pT_t")
                    nc.scalar.activation(pT[:], s_ps[:], Exp, scale=0.125)
                    return pT

                r5_box = [None, None]  # last head's reciprocal + oTc

                def emit_norm(h, o_ps):
                    # drain the accumulator (frees the psum slot):
                    # denominator row + attn rows copied out, then
                    # reciprocal + DRAM broadcast + scale. Copies ride
                    # DVE mid-spine (ACT must keep streaming exps); the
                    # LAST head's copies ride the now-idle ACT since that
                    # chain is the tail's critical path. The reciprocal
                    # stays partition-0-aligned on both sides (custom-DVE
                    # ops don't shift across partitions).
                    hp, u = h // 2, h % 2
                    den = dnp.tile([1, N], F32, tag="dn", name="den_t")
                    oTc = otp.tile([DH, N], BF16, tag="oT", name="oT_t")
                    if h == H - 1:
                        # last head: drain on the now-idle ACT, and
                        # broadcast the reciprocal via a tiny ones-matmul
                        # into PSUM instead of the DRAM round trip — the
                        # two DMA legs plus their completion-semaphore
                        # propagation cost ~5us on this, the tail's
                        # critical chain. (r/ones stay at partition 0;
                        # custom-DVE recip can't shift partitions.)
                        nc.scalar.copy(den[:], o_ps[DH:DH + 1, :])
                        nc.scalar.copy(oTc[:], o_ps[0:DH, :])
                        r_sb = rcp.tile([1, N], F32, tag="r", name="r_t")
                        nc.vector.reciprocal_approx_fast(r_sb[:], den[:])
                        r5_box[0] = r_sb
                        r5_box[1] = oTc
                        return
                    nc.vector.tensor_copy(den[:], o_ps[DH:DH + 1, :])
                    nc.vector.tensor_copy(oTc[:], o_ps[0:DH, :])
                    r_sb = rcp.tile([1, N], F32, tag="r", name="r_t")
                    nc.vector.reciprocal_approx_fast(r_sb[:], den[:])
                    r_dr = dramp.tile([1, N], F32, tag="rdr", name="rdr_t")
                    nc.sync.dma_start(r_dr[:], r_sb[:])
                    rb_sb = rbp.tile([DH, N], F32, tag="rb", name="rb_t")
                    nc.sync.dma_start(rb_sb[:],
                                      r_dr[0:1, :].broadcast_to([DH, N]))
                    nc.vector.tensor_mul(aT[hp][u * DH:(u + 1) * DH, :],
                                         oTc[:], rb_sb[:])

                def emit_pv(h, j, pT):
                    if j == 0:
                        o_ps_box[h] = psumO.tile([P, N], F32, tag="ops",
                                                 name="o_ps")
                    o_ps = o_ps_box[h]
                    for ih in range(2):
                        nc.tensor.matmul(
                            o_ps[:, ih * 512:(ih + 1) * 512],
                            lhsT=vt[j][:, h * P:(h + 1) * P],
                            rhs=pT[:, ih * 512:(ih + 1) * 512],
                            start=(j == 0), stop=(j == NT - 1))
                    if j == NT - 1:
                        emit_norm(h, o_ps)

                # out-projection helpers (some batches weave into the
                # spine's tail slack, so defined before the loop)
                f_tiles = [None] * KT

                def emit_e04(t):
                    f_pool = psum if t % 2 == 1 else psumO
                    f_ps = f_pool.tile([P, N], F32,
                                       tag="ps" if t % 2 == 1 else "ops",
                                       name="f_ps")
                    f_tiles[t] = f_ps
                    for e in range(KT - 1):
                        for ih in range(2):
                            nc.tensor.matmul(
                                f_ps[:, ih * 512:(ih + 1) * 512],
                                lhsT=wo_sb[:, e * D + t * P:
                                           e * D + (t + 1) * P],
                                rhs=aT[e][:, ih * 512:(ih + 1) * 512],
                                start=(e == 0), stop=False)

                def emit_e5_store(t):
                    f_ps = f_tiles[t]
                    for ih in range(2):
                        nc.tensor.matmul(
                            f_ps[:, ih * 512:(ih + 1) * 512],
                            lhsT=wo_sb[:, 5 * D + t * P:5 * D + (t + 1) * P],
                            rhs=aT[5][:, ih * 512:(ih + 1) * 512],
                            start=False, stop=True)
                    o_sb = outp.tile([P, N], BF16, tag="osb", name="osb_t")
                    nc.scalar.add(o_sb[:], f_ps[:], bias_sb[:, t:t + 1])
                    nc.sync.dma_start(out[t * P:(t + 1) * P, :], o_sb[:])

                # 2-step S lead: emit S/exp(g+2) before PV(g), so the S
                # matmuls feeding exp(g+2) run right after exp(g) frees
                # its psum slot instead of queueing behind PV(g) (which
                # itself waits on exp(g)) — keeps the ACT exp stream
                # gapless. PSUM: exactly two S tiles live at any time
                # (the one being exp'd + the one just written). The first
                # e<5 out-proj batch is emitted just before the LAST PV:
                # its psS slot freed at exp(94), so its matmuls fill the
                # PE slack under the final exps instead of the tail.
                steps = [(h, j) for h in range(H) for j in range(NT)]
                pT_q = [emit_s_exp(*steps[0]), emit_s_exp(*steps[1])]
                for gi, st in enumerate(steps):
                    if gi + 2 < len(steps):
                        pT_q.append(emit_s_exp(*steps[gi + 2]))
                    if gi == len(steps) - 1:
                        emit_e04(1)
                    emit_pv(st[0], st[1], pT_q.pop(0))

                # ---- output projection tail, TRANSPOSED: outT[dcol,
                # tok] = sum_e wo(e, dcol-tile)^T @ aT[e] (+ bias via
                # ACT, a per-partition add). e<5 batches first (only e=5
                # needs aT[5], whose head-11 normalize chain is still in
                # flight) — PE is strict FIFO, so a blocked e=5 matmul
                # would stall everything. Batch t=1 was emitted inside
                # the spine already.
                emit_e04(3)
                # last head's reciprocal broadcast: ones-matmul into a
                # psO slot (freed long ago by head 10's drain), then the
                # scale multiply delivers aT[5]'s second half
                rb_ps = psumO.tile([P, N], F32, tag="ops", name="rb_ps")
                r_sb5, oTc5 = r5_box
                for ih in range(2):
                    nc.tensor.matmul(
                        rb_ps[0:DH, ih * 512:(ih + 1) * 512],
                        lhsT=ones_sb[0:1, :],
                        rhs=r_sb5[0:1, ih * 512:(ih + 1) * 512],
                        start=True, stop=True)
                nc.vector.tensor_mul(aT[5][DH:P, :], oTc5[:],
                                     rb_ps[0:DH, :])
                emit_e04(0)
                emit_e04(2)
                emit_e5_store(1)
                emit_e04(5)
                emit_e5_store(3)
                emit_e5_store(0)
                emit_e04(4)
                emit_e5_store(2)
                emit_e5_store(5)
                emit_e5_store(4)

    nc.compile()
    return nc


def _host_tables():
    inv_freq = 1.0 / (10000.0 ** (np.arange(0, DH, 2, dtype=np.float32) / DH))
    t = np.arange(N, dtype=np.float32)
    freqs = np.einsum("i,j->ij", t, inv_freq)          # [N, 32]
    emb = np.concatenate([freqs, freqs], axis=-1)      # [N, 64]
    cosT = np.cos(emb).T.astype(np.float32)            # [64, N]
    sinT = np.sin(emb).T.astype(np.float32)            # [64, N]
    # b-term: out rows 0:32 use -sin (pair d+32), rows 32:64 use +sin.
    # No pre-swap: the device block-swaps qf itself (row-swap DMA), so
    # the sin table stays in output-row order.
    sins = np.concatenate([-sinT[0:32], sinT[32:64]], axis=0)  # [64, N]
    cos2 = np.concatenate([cosT, cosT], axis=0)        # [128, N]
    sins2 = np.concatenate([sins, sins], axis=0)       # [128, N]
    return np.ascontiguousarray(cos2), np.ascontiguousarray(sins2)


def kernel(x, w_qkv, w_out, b_out):
    from concourse.bass_utils import run_bass_kernel_spmd

    if "nc" not in _CACHE:
        _CACHE["nc"] = _build()
    nc = _CACHE["nc"]

    bf = ml_dtypes.bfloat16
    cos2, sins2 = _host_tables()
    cos2 = np.ascontiguousarray(cos2.astype(bf))
    sins2 = np.ascontiguousarray(sins2.astype(bf))
    # [p, t]: bias value for output row t*128+p (transposed out proj)
    biasb = np.ascontiguousarray(
        np.asarray(b_out, np.float32).reshape(KT, P).T)

    def _sbufize(w):   # [(k p), e] -> [p, (k e)] exact SBUF layout
        w = np.asarray(w, np.float32).astype(bf)
        k, e = w.shape[0] // P, w.shape[1]
        return np.ascontiguousarray(
            w.reshape(k, P, e).transpose(1, 0, 2).reshape(P, k * e))

    # q/k weights m-contiguous: [p, (m k 128)] where m = 128-col output
    # tile (12 of them), k = contraction tile — one DMA chunk per m-tile
    wqk_ke = _sbufize(np.asarray(w_qkv, np.float32)[:, 0:1536])  # [p,(k m 128)]
    wqkv_b = np.ascontiguousarray(
        wqk_ke.reshape(P, KT, 12, P).transpose(0, 2, 1, 3).reshape(P, KT * 1536))
    wv_b = _sbufize(np.asarray(w_qkv, np.float32)[:, 1536:E3])
    wout_b = _sbufize(w_out)

    in_maps = []
    for i in range(N_CORES):
        xi = np.ascontiguousarray(
            np.asarray(x[i], np.float32).astype(bf).T)
        in_maps.append({
            "x": xi, "wqkv": wqkv_b, "wv_d": wv_b, "wout": wout_b,
            "cos2": cos2, "sins2": sins2, "biasb": biasb,
        })

    res = run_bass_kernel_spmd(
        nc, in_maps, list(range(N_CORES)),
        trace=bool(int(os.environ.get("KERNEL_TRACE", "0"))))
    _CACHE["last_result"] = res
    return np.stack([np.asarray(res.results[i]["out"]).T
                     for i in range(N_CORES)], axis=0).astype(np.float32)
